# revision 1
# baseline (speedup 1.0000x reference)
"""Two-layer DGL-style GCN on 8 Trainium2 NeuronCores.

Strategy (graph/data parallel, per sharding hint):
- Nodes are sharded 8 ways by destination; each core owns N/8 dst nodes and
  all edges pointing into them (host-side integer preprocessing).
- Each core computes the full projected feature table h = (x * rsqrt(deg_out)) @ W
  locally (input x is replicated), writes it to a local DRAM table, then
  gathers per-edge messages with indirect DMA (128 rows/call, cycled over the
  4 SWDGE queues) and segment-reduces them on the tensor engine: nodes are
  sorted by in-degree into 128-node blocks, chunk t of a block holds every
  node's t-th in-edge, and identity-lhsT matmuls accumulate chunks in PSUM
  (K-restricted to the valid prefix so pad slots contribute zero).
- relu(agg * rsqrt(deg_in) + b) is fused on DVE/ACT; output shards are
  re-assembled and inverse-permuted on the host.
- Layer 2 runs the same compiled NEFF with layer-1's output as input.
"""
import sys, time

sys.path.insert(0, "/opt/trn_rl_repo")
import numpy as np
import jax
from jax.sharding import Mesh, PartitionSpec
from jax.experimental.shard_map import shard_map

import concourse.bass as bass
import concourse.mybir as mybir
import concourse.tile as tile
from concourse.masks import make_identity
from concourse.bass2jax import _bass_exec_p, partition_id_tensor, install_neuronx_cc_hook

P = 128
N_CORES = 8
HDT = None                             # h-table dtype override (None -> fp32)
F = 128                                # feature dim


# ----------------------------------------------------------------------------
# harness plumbing
# ----------------------------------------------------------------------------
def _split_multiwait(nc):
    """This walrus build accepts only one sync-wait per instruction; hoist
    extras onto NoOp carriers placed immediately before."""
    for blk in nc.m.functions[0].blocks:
        new_list, changed = [], False
        for i in list(blk.instructions):
            si = i.sync_info
            if si is not None and si.on_wait and len(si.on_wait) > 1:
                waits = list(si.on_wait)
                for k, w in enumerate(waits[:-1]):
                    c = mybir.InstNoOp(name=f"{i.name}-wsplit{k}", ins=[], outs=[])
                    c.engine = i.engine
                    c.sync_info = mybir.SyncInfo(on_wait=[w], on_update=[])
                    new_list.append(c)
                si.on_wait = [waits[-1]]
                i.sync_info = si
                changed = True
            new_list.append(i)
        if changed:
            blk.instructions = new_list
    return nc


class _Runner:
    def __init__(self, nc, n_cores):
        install_neuronx_cc_hook()
        _split_multiwait(nc)
        self.n_cores = n_cores
        partition_name = nc.partition_id_tensor.name if nc.partition_id_tensor else None
        in_names, out_names, out_avals, zero_outs = [], [], [], []
        for alloc in nc.m.functions[0].allocations:
            if not isinstance(alloc, mybir.MemoryLocationSet):
                continue
            name = alloc.memorylocations[0].name
            if alloc.kind == "ExternalInput":
                if name != partition_name:
                    in_names.append(name)
            elif alloc.kind == "ExternalOutput":
                shape = tuple(alloc.tensor_shape)
                dtype = mybir.dt.np(alloc.dtype)
                out_names.append(name)
                out_avals.append(jax.core.ShapedArray(shape, dtype))
                zero_outs.append(np.zeros(shape, dtype))
        self.in_names, self.out_names = in_names, out_names
        self.out_avals, self.zero_outs = out_avals, zero_outs
        all_in_names = in_names + out_names
        if partition_name is not None:
            all_in_names.append(partition_name)

        def _body(*args):
            operands = list(args)
            if partition_name is not None:
                operands.append(partition_id_tensor())
            outs = _bass_exec_p.bind(
                *operands,
                out_avals=tuple(out_avals),
                in_names=tuple(all_in_names),
                out_names=tuple(out_names),
                lowering_input_output_aliases=(),
                sim_require_finite=False,
                sim_require_nnan=False,
                nc=nc,
            )
            return tuple(outs)

        devices = jax.devices()[:n_cores]
        mesh = Mesh(np.asarray(devices), ("core",))
        n_outs = len(out_names)
        in_specs = (PartitionSpec("core"),) * (len(in_names) + n_outs)
        out_specs = (PartitionSpec("core"),) * n_outs
        self.fn = jax.jit(
            shard_map(_body, mesh=mesh, in_specs=in_specs,
                      out_specs=out_specs, check_rep=False),
            keep_unused=True,
        )

    def run(self, in_maps):
        concat_in = [
            np.concatenate([np.asarray(in_maps[c][n]) for c in range(self.n_cores)], axis=0)
            for n in self.in_names
        ]
        concat_zeros = [
            np.zeros((self.n_cores * z.shape[0], *z.shape[1:]), z.dtype)
            for z in self.zero_outs
        ]
        outs = self.fn(*concat_in, *concat_zeros)
        jax.block_until_ready(outs)
        res = []
        for c in range(self.n_cores):
            m = {}
            for i, name in enumerate(self.out_names):
                m[name] = np.asarray(outs[i]).reshape(
                    self.n_cores, *self.out_avals[i].shape)[c]
            res.append(m)
        return res


# ----------------------------------------------------------------------------
# host-side graph preprocessing
# ----------------------------------------------------------------------------
class _Layout:
    pass


def _prep(edge_src, edge_dst, n_nodes):
    """Per-core padded-CSR layout: nodes sorted by in-degree (desc), grouped
    into 128-node blocks; chunk t of block b holds every node's t-th in-edge
    (pad -> zero row N). Chunk counts per block are shared across cores."""
    N = n_nodes
    SH = N // N_CORES
    NB = (SH + P - 1) // P
    lo = _Layout()
    deg_out = np.bincount(edge_src, minlength=N).astype(np.float32)
    deg_in_g = np.bincount(edge_dst, minlength=N).astype(np.float32)

    per_core = []
    Lb_all = np.zeros((N_CORES, NB), dtype=np.int64)
    for c in range(N_CORES):
        sel = (edge_dst >= c * SH) & (edge_dst < (c + 1) * SH)
        src_c = edge_src[sel]
        dst_c = edge_dst[sel] - c * SH
        counts = np.bincount(dst_c, minlength=SH)
        order_nodes = np.argsort(-counts, kind="stable")      # degree desc
        inv_perm = np.empty(SH, dtype=np.int64)
        inv_perm[order_nodes] = np.arange(SH)
        counts_sorted = counts[order_nodes]
        cs_pad = np.zeros(NB * P, dtype=np.int64)
        cs_pad[:SH] = counts_sorted
        Lb_all[c] = cs_pad.reshape(NB, P).max(axis=1)
        per_core.append((src_c, dst_c, counts, order_nodes, inv_perm))

    Lb = Lb_all.max(axis=0)                                   # common chunk counts
    chunk_base = np.zeros(NB + 1, dtype=np.int64)
    np.cumsum(Lb, out=chunk_base[1:])
    nchunk = int(chunk_base[-1])
    chunk_meta = []                                           # (block, t)
    for b in range(NB):
        for t in range(int(Lb[b])):
            chunk_meta.append((b, t))
    lo.node_tot = NB * P
    lo.nb = NB
    lo.nchunk = nchunk
    lo.chunk_meta = chunk_meta
    lo.lb = Lb

    gidx = np.full((N_CORES, P, nchunk), N, dtype=np.int32)
    degin_t = np.ones((N_CORES, P, NB), dtype=np.float32)
    node_of_pos = np.full((N_CORES, NB * P), -1, dtype=np.int64)
    for c in range(N_CORES):
        src_c, dst_c, counts, order_nodes, inv_perm = per_core[c]
        node_of_pos[c, :SH] = order_nodes + c * SH
        # edge slot assignment, vectorized
        order = np.argsort(dst_c, kind="stable")
        ds = dst_c[order]
        ss = src_c[order]
        starts = np.zeros(SH + 1, dtype=np.int64)
        np.cumsum(counts, out=starts[1:])
        t_idx = np.arange(len(ds)) - starts[ds]               # edge rank within node
        pos = inv_perm[ds]                                    # node position after sort
        blk = pos // P
        prow = pos % P
        q = chunk_base[blk] + t_idx
        gidx[c, prow, q] = ss
        nid = node_of_pos[c]
        valid = nid >= 0
        di = np.ones(NB * P, dtype=np.float32)
        di[valid] = np.maximum(deg_in_g[nid[valid]], 1.0)
        degin_t[c] = di.reshape(NB, P).T

    lo.gidx = gidx
    lo.degin = degin_t
    lo.node_of_pos = node_of_pos
    # common per-chunk valid prefix length (sorted-degree => valid rows are a prefix)
    kq = (gidx != N).sum(axis=1).max(axis=0)               # [nchunk]
    lo.kq = np.maximum(kq, 2).astype(np.int64)
    nproj = (N + P - 1) // P
    dout = np.ones(nproj * P, dtype=np.float32)
    dout[:N] = np.maximum(deg_out, 1.0)
    lo.degout = dout.reshape(nproj, P).T.copy()
    lo.nproj = nproj
    lo.n = N
    return lo


# ----------------------------------------------------------------------------
# device kernel
# ----------------------------------------------------------------------------
def _build_nc(lo, repeat=1):
    N, NPROJ, NB, NCHUNK = lo.n, lo.nproj, lo.nb, lo.nchunk
    NODE_TOT = lo.node_tot
    nc = bass.Bass(num_swdge_queues=4)
    tc = tile.TileContext(nc)
    f32 = mybir.dt.float32
    hdt = HDT or f32

    xT = nc.dram_tensor("xT", [P, NPROJ * P], f32, kind="ExternalInput")
    W = nc.dram_tensor("W", [P, F], f32, kind="ExternalInput")
    brow = nc.dram_tensor("brow", [1, F], f32, kind="ExternalInput")
    degout = nc.dram_tensor("degout", [P, NPROJ], f32, kind="ExternalInput")
    degin = nc.dram_tensor("degin", [P, NB], f32, kind="ExternalInput")
    gidx = nc.dram_tensor("gidx", [P, NCHUNK], mybir.dt.int32, kind="ExternalInput")
    out = nc.dram_tensor("out", [NODE_TOT, F], f32, kind="ExternalOutput")
    h_table = nc.dram_tensor("h_table", [N + 1, F], hdt)

    XCH = 4096                       # xT columns per load chunk

    with tc:
        with (
            tc.tile_pool(name="const", bufs=1) as constp,
            tc.tile_pool(name="xin", bufs=2) as xinp,
            tc.tile_pool(name="hsb", bufs=4) as hsbp,
            tc.tile_pool(name="msg", bufs=32) as msgp,
            tc.tile_pool(name="osb", bufs=4) as osbp,
            tc.tile_pool(name="ppsum", bufs=3, space="PSUM") as ppsum,
            tc.tile_pool(name="apsum", bufs=4, space="PSUM") as apsum,
            tc.tile_pool(name="bpsum", bufs=1, space="PSUM") as bpsum,
        ):
            # ---- constants
            W_sb = constp.tile([P, F], f32)
            nc.sync.dma_start(W_sb[:], W[:])
            ident_f = constp.tile([P, P], f32)
            make_identity(nc, ident_f[:])
            ident = constp.tile([P, P], hdt)
            nc.vector.tensor_copy(ident[:], ident_f[:])
            gidx_sb = constp.tile([P, NCHUNK], mybir.dt.int32)
            nc.sync.dma_start(gidx_sb[:], gidx[:])
            brow_sb = constp.tile([1, F], f32)
            nc.sync.dma_start(brow_sb[:], brow[:])

            do_sb = constp.tile([P, NPROJ], f32)
            nc.sync.dma_start(do_sb[:], degout[:])
            do_rs = constp.tile([P, NPROJ], f32)
            nc.scalar.activation(do_rs[:], do_sb[:], mybir.ActivationFunctionType.Sqrt)
            nc.vector.reciprocal(do_rs[:], do_rs[:])

            di_sb = constp.tile([P, NB], f32)
            nc.sync.dma_start(di_sb[:], degin[:])
            di_rs = constp.tile([P, NB], f32)
            nc.scalar.activation(di_rs[:], di_sb[:], mybir.ActivationFunctionType.Sqrt)
            nc.vector.reciprocal(di_rs[:], di_rs[:])

            ones1 = constp.tile([1, F], f32)
            nc.vector.memset(ones1[:], 1.0)
            bps = bpsum.tile([P, F], f32)
            nc.tensor.matmul(out=bps[:], lhsT=ones1[:], rhs=brow_sb[:],
                             start=True, stop=True)
            b_bcast = constp.tile([P, F], f32)
            nc.vector.tensor_copy(b_bcast[:], bps[:])

            zrow = constp.tile([1, F], hdt)
            nc.vector.memset(zrow[:], 0.0)
            nc.sync.dma_start(h_table[N:N + 1, :], zrow[:])

            for _rep in range(repeat):
                # ---- projection: h_table[n] = (x[n] * rs_out[n]) @ W
                n_xch = (NPROJ * P + XCH - 1) // XCH
                tglob = 0
                for xc in range(n_xch):
                    cols = min(XCH, NPROJ * P - xc * XCH)
                    xt = xinp.tile([P, XCH], f32)
                    nc.sync.dma_start(xt[:, :cols], xT[:, xc * XCH:xc * XCH + cols])
                    for t in range(cols // P):
                        m = min(P, N - tglob * P)
                        if m <= 0:
                            break
                        pp = ppsum.tile([P, F], f32)
                        nc.tensor.matmul(out=pp[:m, :], lhsT=xt[:, t * P:t * P + m],
                                         rhs=W_sb[:], start=True, stop=True)
                        hs = hsbp.tile([P, F], hdt)
                        if tglob % 2 == 0:
                            nc.vector.tensor_scalar(
                                hs[:m, :], pp[:m, :], do_rs[:m, tglob:tglob + 1], None,
                                mybir.AluOpType.mult)
                        else:
                            nc.scalar.activation(
                                hs[:m, :], pp[:m, :], mybir.ActivationFunctionType.Copy,
                                scale=do_rs[:m, tglob:tglob + 1])
                        nc.sync.dma_start(h_table[tglob * P:tglob * P + m, :], hs[:m, :])
                        tglob += 1

                # ---- gather + segment-reduce + output

                def flush_block(b, agg):
                    o1 = osbp.tile([P, F], f32)
                    nc.vector.tensor_scalar(o1[:], agg[:], di_rs[:, b:b + 1], None,
                                            mybir.AluOpType.mult)
                    nc.vector.tensor_tensor(o1[:], o1[:], b_bcast[:],
                                            op=mybir.AluOpType.add)
                    o2 = osbp.tile([P, F], f32)
                    nc.scalar.activation(o2[:], o1[:], mybir.ActivationFunctionType.Relu)
                    nc.sync.dma_start(out[b * P:(b + 1) * P, :], o2[:])

                agg = None
                for q in range(NCHUNK):
                    b, t = lo.chunk_meta[q]
                    kt = int(lo.kq[q])
                    mt = msgp.tile([P, F], hdt)
                    inst = nc.gpsimd.indirect_dma_start(
                        out=mt[:kt, :],
                        out_offset=None,
                        in_=h_table[:],
                        in_offset=bass.IndirectOffsetOnAxis(
                            ap=gidx_sb[:kt, q:q + 1], axis=0),
                    )
                    qi = q % 4
                    inst.ins.queue = f"qPoolDynamic{qi if qi else ''}"
                    if t == 0:
                        agg = apsum.tile([P, F], f32)
                    last = (t == int(lo.lb[b]) - 1)
                    nc.tensor.matmul(
                        out=agg[:],
                        lhsT=ident[:kt, :],
                        rhs=mt[:kt, :],
                        start=(t == 0), stop=last)
                    if last:
                        flush_block(b, agg)
                # blocks with no chunks (all-dummy): write relu(b)
                for b in range(NB):
                    if int(lo.lb[b]) == 0:
                        agg = apsum.tile([P, F], f32)
                        nc.vector.memset(agg[:], 0.0)
                        flush_block(b, agg)
    return nc



# ----------------------------------------------------------------------------
# public entry
# ----------------------------------------------------------------------------
_CACHE = {}


def _get_runner(edge_src, edge_dst, n_nodes):
    key = (n_nodes, edge_src.shape[0],
           int(edge_src[::997].astype(np.int64).sum()),
           int(edge_dst[::997].astype(np.int64).sum()))
    if key not in _CACHE:
        lo = _prep(edge_src, edge_dst, n_nodes)
        nc = _build_nc(lo)
        _CACHE[key] = (lo, _Runner(nc, N_CORES))
    return _CACHE[key]


def _layer(runner, lo, x, W, b):
    N = lo.n
    xt_full = np.zeros((P, lo.nproj * P), dtype=np.float32)
    xt_full[:, :N] = np.ascontiguousarray(x.T)
    in_maps = []
    for c in range(N_CORES):
        in_maps.append({
            "xT": xt_full,
            "W": np.ascontiguousarray(W.astype(np.float32)),
            "brow": np.ascontiguousarray(b.astype(np.float32)[None, :]),
            "degout": lo.degout,
            "degin": lo.degin[c],
            "gidx": lo.gidx[c],
        })
    res = runner.run(in_maps)
    out_full = np.zeros((N, F), dtype=np.float32)
    for c in range(N_CORES):
        nid = lo.node_of_pos[c]
        valid = nid >= 0
        out_full[nid[valid]] = res[c]["out"][valid]
    return out_full


def kernel(features, edge_src, edge_dst, W1, b1, W2, b2):
    features = np.asarray(features, dtype=np.float32)
    edge_src = np.asarray(edge_src, dtype=np.int32)
    edge_dst = np.asarray(edge_dst, dtype=np.int32)
    n = features.shape[0]
    lo, runner = _get_runner(edge_src, edge_dst, n)
    h1 = _layer(runner, lo, features, np.asarray(W1), np.asarray(b1))
    h2 = _layer(runner, lo, h1, np.asarray(W2), np.asarray(b2))
    return h2

